# revision 8
# baseline (speedup 1.0000x reference)
"""
Trainium2 Bass kernel for nn_DenseFeatureNumericEmbedding.

Computes, per feature f (F=128 independent tiny MLPs):
    h[b,f,:]   = relu(x[b,f] * w1[f,:] + b1[f,:])            # [B, F, H]
    out[b,f,:] = h[b,f,:] @ w2[f,:,:] + b2[f,:]              # [B, F, E]
    returns out.reshape(B, F*E)                              # [16384, 4096] fp32

Sharding: data-parallel over batch across 8 NeuronCores (2048 rows/core),
params replicated. No collectives; host concatenates the 8 output shards.

Per-core dataflow (per 512-batch chunk, per quad of 4 features):
  L1   TensorE: K=2 matmuls, stationary [w1[f]; b1[f]], moving [xT[f]; ones]
       -> preactT [H=128, 512] in PSUM (bias folded into the matmul).
       The 4 features of a quad use row-groups 0..3 (tile_position) so their
       matmuls run concurrently in the 128x128 array.
  RELU ScalarE activation(Relu) / VectorE tensor_scalar_max(0) split,
       PSUM -> SBUF, cast to bf16 -> hT [128, 2048].
  L2   TensorE: per feature, stationary w2[f] [H,E], moving hT -> col-tiled
       4 features into one PSUM bank -> outT [FE=128, 512].
  B2+COPY ScalarE activation(Identity, bias=b2 column) PSUM -> SBUF fp32.
  TRANS TensorE transpose (fp32) -> PSUM [b, fe], VectorE copy -> SBUF.
  DMA  per-quad store, 512B contiguous runs in DRAM.
"""

import sys

sys.path.insert(0, "/opt/trn_rl_repo")

import numpy as np
import ml_dtypes

import concourse.bass as bass
import concourse.tile as tile
from concourse import bacc, mybir
from concourse.bass_utils import run_bass_kernel_spmd

BF16 = ml_dtypes.bfloat16

B = 16384
F = 128
H = 128
E = 32
NCORES = 8
BL = B // NCORES          # 2048 rows per core
CHUNK = 512               # batch columns per inner tile (1 PSUM bank fp32)
NCHUNK = BL // CHUNK      # 4
NQUAD = F // 4            # 32 quads of 4 features

# Fraction of relu half-quads (FD=1024 instrs) assigned to ScalarE; the rest
# go to VectorE.  Out of every 8 halves, this many go to ACT.
ACT_OF_8 = 5

_COMPILED = None


def _build_bass():
    nc = bacc.Bacc("TRN2", target_bir_lowering=False, debug=False,
                   num_devices=NCORES)
    dt = mybir.dt

    xt2 = nc.dram_tensor("xt2", [2 * F, BL], dt.bfloat16, kind="ExternalInput").ap()
    w1b1q = nc.dram_tensor("w1b1q", [128, F * H], dt.bfloat16, kind="ExternalInput").ap()
    w2s = nc.dram_tensor("w2s", [H, F * E], dt.bfloat16, kind="ExternalInput").ap()
    b2qs = nc.dram_tensor("b2qs", [128, NQUAD], dt.float32, kind="ExternalInput").ap()
    eye = nc.dram_tensor("eye", [128, 128], dt.float32, kind="ExternalInput").ap()
    out = nc.dram_tensor("out", [BL, F * E], dt.float32, kind="ExternalOutput").ap()

    # DRAM views
    # xt2 rows: 8q + 2j + r  (q quad, j feature-in-quad, r 0=x / 1=ones)
    xt2_r = xt2.rearrange("(q g) n -> g q n", g=8)       # [8, NQUAD, BL]
    # out rows: 512c + 128jj + p
    out_r = out.rearrange("(c jj p) n -> c p jj n", jj=4, p=128)  # [NCHUNK,128,4,FE]

    with tile.TileContext(nc) as tc:
        with (
            tc.tile_pool(name="params", bufs=1) as params,
            tc.tile_pool(name="xq", bufs=2) as xq_pool,
            tc.tile_pool(name="h", bufs=4) as h_pool,
            tc.tile_pool(name="outT", bufs=4) as outT_pool,
            tc.tile_pool(name="outq", bufs=4) as outq_pool,
            tc.tile_pool(name="pre", bufs=2, space="PSUM") as pre_pool,
            tc.tile_pool(name="pout", bufs=2, space="PSUM") as pout_pool,
            tc.tile_pool(name="ptr", bufs=2, space="PSUM") as ptr_pool,
        ):
            w1b1q_sb = params.tile([128, F * H], dt.bfloat16, tag="w1b1q")
            nc.sync.dma_start(out=w1b1q_sb[:], in_=w1b1q[:])
            w2_sb = params.tile([H, F * E], dt.bfloat16, tag="w2s")
            nc.sync.dma_start(out=w2_sb[:], in_=w2s[:])
            b2_sb = params.tile([128, NQUAD], dt.float32, tag="b2qs")
            nc.sync.dma_start(out=b2_sb[:], in_=b2qs[:])
            eye_sb = params.tile([128, 128], dt.float32, tag="eye")
            nc.sync.dma_start(out=eye_sb[:], in_=eye[:])

            relu_idx = 0
            for c in range(NCHUNK):
                # xq[32j + r, 512q + cc] = xt2[8q + 2j + r, 512c + cc]
                xq = xq_pool.tile([128, NQUAD * CHUNK], dt.bfloat16, tag="xq")
                for j in range(4):
                    nc.sync.dma_start(
                        out=xq[32 * j:32 * j + 2, :].rearrange(
                            "r (q n) -> r q n", n=CHUNK),
                        in_=xt2_r[2 * j:2 * j + 2, :,
                                  bass.ts(c, CHUNK)],
                    )

                for q in range(NQUAD):
                    # ---- L1: 4 features, row-groups 0..3, K=2 matmuls ----
                    pre_a = pre_pool.tile([128, 2 * CHUNK], dt.float32, tag="pre")
                    pre_b = pre_pool.tile([128, 2 * CHUNK], dt.float32, tag="pre")
                    for j in range(4):
                        tgt = pre_a if j < 2 else pre_b
                        nc.tensor.matmul(
                            tgt[:, bass.ts(j % 2, CHUNK)],
                            lhsT=w1b1q_sb[32 * j:32 * j + 2, bass.ts(q, H)],
                            rhs=xq[32 * j:32 * j + 2, bass.ts(q, CHUNK)],
                            start=True, stop=True,
                            tile_position=(32 * j, 0),
                        )

                    # ---- relu + cast bf16, split ACT / DVE ----
                    hT = h_pool.tile([128, 4 * CHUNK], dt.bfloat16, tag="h")
                    for half, hsrc in ((0, pre_a), (1, pre_b)):
                        dst = hT[:, bass.ts(half, 2 * CHUNK)]
                        if relu_idx % 8 < ACT_OF_8:
                            nc.scalar.activation(
                                dst, hsrc[:], mybir.ActivationFunctionType.Relu)
                        else:
                            nc.vector.tensor_scalar_max(dst, hsrc[:], 0.0)
                        relu_idx += 1

                    # ---- L2: 4 features col-tiled into one PSUM bank ----
                    pout = pout_pool.tile([128, CHUNK], dt.float32, tag="pout")
                    for j in range(4):
                        f = 4 * q + j
                        nc.tensor.matmul(
                            pout[32 * j:32 * j + 32, :],
                            lhsT=w2_sb[:, bass.ts(f, E)],
                            rhs=hT[:, bass.ts(j, CHUNK)],
                            start=True, stop=True,
                            tile_position=(0, 32 * j),
                        )

                    # ---- + b2, PSUM -> SBUF fp32 ----
                    outT = outT_pool.tile([128, CHUNK], dt.float32, tag="outT")
                    nc.scalar.activation(
                        outT[:], pout[:],
                        mybir.ActivationFunctionType.Identity,
                        bias=b2_sb[:, q:q + 1],
                    )

                    # ---- transpose [fe, b] -> [b, fe] via TensorE ----
                    ptr = ptr_pool.tile([128, CHUNK], dt.float32, tag="ptr")
                    for jj in range(4):
                        nc.tensor.transpose(
                            ptr[:, bass.ts(jj, 128)],
                            outT[:, bass.ts(jj, 128)],
                            eye_sb[:],
                        )

                    outq = outq_pool.tile([128, CHUNK], dt.float32, tag="outq")
                    nc.vector.tensor_copy(outq[:], ptr[:])

                    # ---- store: rows 512c+128jj+p, cols 128q..128q+128 ----
                    nc.sync.dma_start(
                        out=out_r[c, :, :, bass.ts(q, 128)],
                        in_=outq[:].rearrange("p (jj n) -> p jj n", n=128),
                    )

    nc.compile()
    return nc


def _prep_inputs(x, w1, b1, w2, b2):
    """Host-side packing of parameters + per-core x shards."""
    w1b1q = np.zeros((128, F * H), dtype=BF16)
    for f in range(F):
        q, j = divmod(f, 4)
        w1b1q[32 * j + 0, H * q:H * q + H] = w1[f].astype(BF16)
        w1b1q[32 * j + 1, H * q:H * q + H] = b1[f].astype(BF16)

    w2s = np.ascontiguousarray(
        w2.transpose(1, 0, 2).reshape(H, F * E)).astype(BF16)
    # b2qs[32j + e, q] = b2[4q + j, e]
    b2qs = np.ascontiguousarray(
        b2.reshape(NQUAD, 4, E).transpose(1, 2, 0).reshape(128, NQUAD)
    ).astype(np.float32)
    eye = np.eye(128, dtype=np.float32)

    in_maps = []
    for core in range(NCORES):
        xs = x[core * BL:(core + 1) * BL]          # [BL, F]
        xt2 = np.empty((2 * F, BL), dtype=BF16)
        xt2[0::2] = xs.T.astype(BF16)
        xt2[1::2] = BF16(1.0)
        in_maps.append({
            "xt2": xt2, "w1b1q": w1b1q, "w2s": w2s,
            "b2qs": b2qs, "eye": eye,
        })
    return in_maps


def _get_compiled():
    global _COMPILED
    if _COMPILED is None:
        _COMPILED = _build_bass()
    return _COMPILED


def kernel(x, w1, b1, w2, b2, _trace=False, _trace_kwargs=None):
    nc = _get_compiled()
    in_maps = _prep_inputs(
        np.asarray(x, dtype=np.float32), np.asarray(w1, dtype=np.float32),
        np.asarray(b1, dtype=np.float32), np.asarray(w2, dtype=np.float32),
        np.asarray(b2, dtype=np.float32))
    res = run_bass_kernel_spmd(
        nc, in_maps, core_ids=list(range(NCORES)),
        trace=_trace, **(_trace_kwargs or {}))
    shards = [np.asarray(res.results[i]["out"]) for i in range(NCORES)]
    full = np.concatenate(shards, axis=0).astype(np.float32)
    if _trace:
        return full, res
    return full


if __name__ == "__main__":
    rng = np.random.default_rng(0)
    x = rng.standard_normal((B, F), dtype=np.float32)
    w1 = rng.standard_normal((F, H), dtype=np.float32)
    b1 = rng.standard_normal((F, H), dtype=np.float32)
    w2 = (rng.standard_normal((F, H, E), dtype=np.float32) / np.sqrt(H)).astype(np.float32)
    b2 = rng.standard_normal((F, E), dtype=np.float32) / np.sqrt(H)
    got = kernel(x=x, w1=w1, b1=b1, w2=w2, b2=b2)
    h = np.maximum(x[:, :, None] * w1[None] + b1[None], 0.0)
    want = (np.einsum("bfh,fhe->bfe", h, w2) + b2[None]).reshape(B, F * E)
    err = np.abs(got - want).max() / np.abs(want).max()
    print("self-test scale-relative max err:", err)
